# revision 32
# baseline (speedup 1.0000x reference)
"""Trainium2 Bass kernel for nn_Attention_37598143710100.

Full attention layer: qkv proj -> rms norm q,k -> rope -> softmax attention
-> out proj.  B=4, L=4096, C=1024, H=16, D=64.

Sharding: 8 cores = (batch b in 0..3) x (query half qh in 0..1).  Each core
computes out[b, qh*2048:(qh+1)*2048, :] completely; host concatenates.
Key/value positions are permuted to [own-half | other-half] so the SPMD
program is identical across cores (softmax is order-invariant).

v3: single fused software pipeline.  The softmax exp on ScalarE (134M
elements/core ~ 1.02ms) is the hard floor; everything else is scheduled to
hide beneath it.  Projection work for head-pair hp+1 is emitted interleaved
into the attention slot loop of hp so TensorE never idles (keeps PE HAM
warm at 2.4GHz).  PSUM statically partitioned: scores 4 banks (double-
buffered [128,1024]), ctx accumulator 2 banks (head-serial attention),
projection 1 bank, rms-sums 1 bank.  Scores are nh-packed via duplicated
khat/qhat so one head's two N=512 matmuls run concurrently in disjoint
row groups.  V is transposed with the DMA xbar instead of TensorE.
Out-projection interleaves into the last head-pair's attention.
"""

import numpy as np
import ml_dtypes

B, L, C, H, D = 4, 4096, 1024, 16, 64
NCORES = 8
LQ = L // 2
EPS = 1e-6
NPAIR = H // 2
RSQ_A, RSQ_B = 1.3750, 0.2700  # rsqrt Newton init y0 = A - B*x on [0.4, 3.5]

_compiled = None


def _build():
    import concourse.tile as tile
    from concourse import bacc, mybir
    from concourse.masks import make_identity

    bf16 = mybir.dt.bfloat16
    f32 = mybir.dt.float32
    AF = mybir.ActivationFunctionType

    nc = bacc.Bacc("TRN2", target_bir_lowering=False, debug=False,
                   enable_asserts=True, num_devices=NCORES)

    xT = nc.dram_tensor("xT", [C, L], bf16, kind="ExternalInput").ap()
    wT = nc.dram_tensor("wT", [C, 3 * C], bf16, kind="ExternalInput").ap()
    wpT = nc.dram_tensor("wpT", [C, C], bf16, kind="ExternalInput").ap()
    cgq = nc.dram_tensor("cgq", [D, LQ], bf16, kind="ExternalInput").ap()
    sgq = nc.dram_tensor("sgq", [D, LQ], bf16, kind="ExternalInput").ap()
    cgk = nc.dram_tensor("cgk", [D, L], bf16, kind="ExternalInput").ap()
    sgk = nc.dram_tensor("sgk", [D, L], bf16, kind="ExternalInput").ap()
    onesAB = nc.dram_tensor("onesAB", [128, 2], bf16, kind="ExternalInput").ap()
    bproj = nc.dram_tensor("bproj", [1, C], f32, kind="ExternalInput").ap()
    out_ap = nc.dram_tensor("out", [LQ, C], f32, kind="ExternalOutput").ap()

    # scratch: rms inv rows (q: 2jc+h, k: 8+2jc+h), softmax recips, ctx bounce
    inv_sc = nc.dram_tensor("inv_sc", [NPAIR, 24, 512], bf16).ap()
    rcp_sc = nc.dram_tensor("rcp_sc", [NPAIR, 2, 2, 1024], bf16).ap()
    ctx_sc = nc.dram_tensor("ctx_sc", [NPAIR, 128, LQ], bf16).ap()

    xTr = xT.rearrange("(eb p) j -> p eb j", p=128)
    wTr = wT.rearrange("(eb p) f -> p eb f", p=128)
    wpTr = wpT.rearrange("(cb p) o -> p cb o", p=128)

    from contextlib import ExitStack
    with tile.TileContext(nc) as tc:
        with ExitStack() as stack:
            persist = stack.enter_context(tc.tile_pool(name="persist", bufs=1))
            pairq = stack.enter_context(tc.tile_pool(name="pairq", bufs=2))
            pairk = stack.enter_context(tc.tile_pool(name="pairk", bufs=2))
            pairv = stack.enter_context(tc.tile_pool(name="pairv", bufs=2))
            trans = stack.enter_context(tc.tile_pool(name="trans", bufs=1))
            wslp = stack.enter_context(tc.tile_pool(name="wslp", bufs=1))
            work = stack.enter_context(tc.tile_pool(name="work", bufs=2))
            work1 = stack.enter_context(tc.tile_pool(name="work1", bufs=1))
            xep = stack.enter_context(tc.tile_pool(name="xe", bufs=8))
            vtcp = stack.enter_context(tc.tile_pool(name="vtc", bufs=2))
            exps = stack.enter_context(tc.tile_pool(name="exps", bufs=2))
            outp = stack.enter_context(tc.tile_pool(name="outp", bufs=2))
            otp = stack.enter_context(tc.tile_pool(name="otp", bufs=1))
            att = stack.enter_context(
                tc.tile_pool(name="att", bufs=2, space="PSUM"))
            cxp = stack.enter_context(
                tc.tile_pool(name="cxp", bufs=1, space="PSUM"))
            ppp = stack.enter_context(
                tc.tile_pool(name="ppp", bufs=1, space="PSUM"))
            ps2 = stack.enter_context(
                tc.tile_pool(name="ps2", bufs=1, space="PSUM"))

            onesT = persist.tile([128, 2], bf16, tag="onesT")
            nc.sync.dma_start(onesT[:], onesAB[:])

            cgq_b = persist.tile([128, LQ], bf16, tag="cgq")
            sgq_b = persist.tile([128, LQ], bf16, tag="sgq")
            cgk_b = persist.tile([128, L], bf16, tag="cgk")
            sgk_b = persist.tile([128, L], bf16, tag="sgk")
            for t, src in ((cgq_b, cgq), (sgq_b, sgq), (cgk_b, cgk), (sgk_b, sgk)):
                nc.sync.dma_start(t[0:64, :], src[:])
                nc.sync.dma_start(t[64:128, :], src[:])

            wp_sb = persist.tile([128, 8, C], bf16, tag="wp")
            nc.sync.dma_start(wp_sb[:], wpTr[:])
            bp_b = persist.tile([128, C], f32, tag="bp")
            nc.sync.dma_start(bp_b[:], bproj[0:1, :].partition_broadcast(128))
            ones_col = persist.tile([128, 1], bf16, tag="onescol")
            nc.vector.memset(ones_col[:], 1.0)
            ident = persist.tile([128, 128], bf16, tag="ident")
            make_identity(nc, ident[:])

            # ---------------- projection work-item generator ----------------
            # Emits the full qkv projection + rms + rope + dup for head-pair
            # hp as a list of closures; the attention loop of hp-1 drains
            # them a few per slot so PE/DVE/DMA stay dense while ACT runs.
            def proj_items(hp, head=False):
                st = {}
                items = []
                # at the pipeline head the attention psum banks are idle;
                # cycle chains through them so projections run in parallel
                chain_pools = ([(ppp, "pp"), (att, "att"), (att, "att"),
                                (cxp, "cxp")] if head else [(ppp, "pp")])
                st['ci'] = 0

                def mk_wsl():
                    w_sl = wslp.tile([128, 8, 3, 128], bf16, tag="wsl",
                                     name=f"wsl{hp}")
                    for t in range(3):
                        nc.sync.dma_start(
                            w_sl[:, :, t, :],
                            wTr[:, :, t * C + hp * 128: t * C + (hp + 1) * 128])
                    st['w'] = w_sl
                    st['qraw'] = trans.tile([128, LQ], bf16, tag="qraw",
                                            name=f"qraw{hp}")
                    st['kraw'] = trans.tile([128, L], bf16, tag="kraw",
                                            name=f"kraw{hp}")
                    st['qshf'] = trans.tile([128, LQ], bf16, tag="qshf",
                                            name=f"qshf{hp}")
                    st['kshf'] = trans.tile([128, L], bf16, tag="kshf",
                                            name=f"kshf{hp}")
                    st['coll'] = trans.tile([24, 512], bf16, tag="coll",
                                            name=f"coll{hp}")
                    st['vsb'] = pairv.tile([128, 32, 2, 65], bf16, tag="vsb",
                                           name=f"vsb{hp}")
                    nc.vector.memset(st['vsb'][:, :, :, 64:65], 1.0)
                items.append(mk_wsl)

                def mk_xe(jc, eb):
                    def go():
                        t = xep.tile([128, 512], bf16, tag="xe",
                                     name=f"xe{hp}_{jc}_{eb}")
                        nc.gpsimd.dma_start(t[:], xTr[:, eb, jc * 512:(jc + 1) * 512])
                        st[('xe', eb)] = t
                    return go

                def mk_chain(jc, tsr):
                    # one psum chain: 8 accumulating MMs -> epilogue
                    key = ('pp', jc, tsr)

                    def go_mm(eb):
                        def f():
                            if eb == 0:
                                pool, tag = chain_pools[st['ci']
                                                        % len(chain_pools)]
                                st['ci'] += 1
                                st[key] = pool.tile(
                                    [128, 512], f32, tag=tag,
                                    name=f"pp{hp}_{jc}_{tsr}")
                            nc.tensor.matmul(st[key][:],
                                             st['w'][:, eb, tsr, :],
                                             st[('xe', eb)][:],
                                             start=(eb == 0), stop=(eb == 7))
                        return f

                    def go_epi():
                        ps = st[key]
                        sl = slice(jc * 512, (jc + 1) * 512)
                        if tsr == 2:  # v: cast; PE-transposes follow as items
                            vTc = vtcp.tile([128, 512], bf16, tag="vtc",
                                            name=f"vtc{hp}_{jc}")
                            nc.vector.tensor_copy(vTc[:], ps[:])
                            st['vtc'] = vTc
                        else:
                            raw = st['qraw'] if tsr == 0 else st['kraw']
                            nc.vector.tensor_copy(raw[:, sl], ps[:])
                            sq = work1.tile([128, 512], bf16, tag="sq",
                                            name=f"sq{hp}_{jc}_{tsr}")
                            nc.vector.tensor_mul(sq[:], raw[:, sl], raw[:, sl])
                            pss = ps2.tile([2, 512], f32, tag="pss",
                                           name=f"pss{hp}_{jc}_{tsr}")
                            nc.tensor.matmul(pss[:], onesT[:], sq[:],
                                             start=True, stop=True)
                            cp2 = work1.tile([2, 512], bf16, tag="cp2",
                                             name=f"cp2{hp}_{jc}_{tsr}")
                            nc.vector.tensor_copy(cp2[:], pss[:])
                            r0 = 2 * jc if tsr == 0 else 8 + 2 * jc
                            nc.sync.dma_start(st['coll'][r0:r0 + 2, :], cp2[:])
                    return [go_mm(eb) for eb in range(8)] + [go_epi]

                def mk_vtrans(jc, jt):
                    # PE transpose of one [128,128] vTc chunk into vsb,
                    # borrowing the proj psum slot
                    def go():
                        jg = jc * 4 + jt
                        tp = ppp.tile([128, 128], bf16, tag="pp",
                                      name=f"tp{hp}_{jg}")
                        nc.tensor.transpose(
                            tp[:], st['vtc'][:, jt * 128:(jt + 1) * 128],
                            ident[:])
                        nc.vector.tensor_copy(
                            st['vsb'][:, jg, :, 0:64],
                            tp[:].rearrange("p (h d) -> p h d", h=2))
                    return go

                for jc in range(8):
                    for eb in range(8):
                        items.append(mk_xe(jc, eb))
                    items.extend(mk_chain(jc, 1))
                    if jc < 4:
                        items.extend(mk_chain(jc, 0))
                    items.extend(mk_chain(jc, 2))
                    items.extend(mk_vtrans(jc, jt) for jt in range(4))

                def mk_shifts():
                    for raw, shf in ((st['qraw'], st['qshf']),
                                     (st['kraw'], st['kshf'])):
                        nc.sync.dma_start(shf[0:32, :], raw[32:64, :])
                        nc.sync.dma_start(shf[32:64, :], raw[0:32, :])
                        nc.sync.dma_start(shf[64:96, :], raw[96:128, :])
                        nc.sync.dma_start(shf[96:128, :], raw[64:96, :])
                items.append(mk_shifts)

                def mk_newton():
                    # rsqrt(ms+eps) via Newton on DVE: x = coll/64 + eps
                    xms = work1.tile([24, 512], f32, tag="xms", name=f"xms{hp}")
                    nc.vector.tensor_scalar(
                        xms[:], st['coll'][:], 1.0 / 64.0, EPS,
                        op0=mybir.AluOpType.mult, op1=mybir.AluOpType.add)
                    y = work1.tile([24, 512], f32, tag="y", name=f"y{hp}")
                    nc.vector.tensor_scalar(
                        y[:], xms[:], -RSQ_B, RSQ_A,
                        op0=mybir.AluOpType.mult, op1=mybir.AluOpType.add)
                    t1 = work1.tile([24, 512], f32, tag="t1", name=f"t1{hp}")
                    t2 = work1.tile([24, 512], f32, tag="t2", name=f"t2{hp}")
                    inv24 = work1.tile([24, 512], bf16, tag="inv24",
                                       name=f"inv24{hp}")
                    for it in range(3):
                        nc.vector.tensor_mul(t1[:], xms[:], y[:])
                        nc.vector.tensor_mul(t2[:], t1[:], y[:])
                        nc.vector.tensor_scalar(
                            t2[:], t2[:], -0.5, 1.5,
                            op0=mybir.AluOpType.mult, op1=mybir.AluOpType.add)
                        if it < 2:
                            nc.vector.tensor_mul(y[:], y[:], t2[:])
                        else:
                            nc.vector.tensor_mul(inv24[:], y[:], t2[:])
                    nc.sync.dma_start(inv_sc[hp, :, :], inv24[:])
                items.append(mk_newton)

                def mk_hat(side):
                    t = trans.tile([128, LQ if side == 'q' else L], bf16,
                                   tag=f"hat{side}", name=f"hat{side}{hp}")
                    st[f'hat{side}'] = t
                items.append(lambda: mk_hat('q'))
                items.append(lambda: mk_hat('k'))

                def mk_rope(side, jc):
                    # hat = raw*inv*cg + shf*inv*sg for one 512 chunk
                    def go():
                        raw = st['qraw'] if side == 'q' else st['kraw']
                        shf = st['qshf'] if side == 'q' else st['kshf']
                        cg_b = cgq_b if side == 'q' else cgk_b
                        sg_b = sgq_b if side == 'q' else sgk_b
                        hat = st[f'hat{side}']
                        r0 = 0 if side == 'q' else 8
                        sl = slice(jc * 512, (jc + 1) * 512)
                        rA = r0 + 2 * jc
                        ib = work.tile([128, 512], bf16, tag="ib",
                                       name=f"ib{hp}{side}{jc}")
                        nc.sync.dma_start(
                            ib[0:64, :],
                            inv_sc[hp, rA:rA + 1, :].partition_broadcast(64))
                        nc.sync.dma_start(
                            ib[64:128, :],
                            inv_sc[hp, rA + 1:rA + 2, :].partition_broadcast(64))
                        icg = work.tile([128, 512], bf16, tag="icg",
                                        name=f"icg{hp}{side}{jc}")
                        nc.vector.tensor_mul(icg[:], ib[:], cg_b[:, sl])
                        isg = work.tile([128, 512], bf16, tag="isg",
                                        name=f"isg{hp}{side}{jc}")
                        nc.vector.tensor_mul(isg[:], ib[:], sg_b[:, sl])
                        u = work.tile([128, 512], bf16, tag="u",
                                      name=f"u{hp}{side}{jc}")
                        nc.vector.tensor_mul(u[:], raw[:, sl], icg[:])
                        v2 = work.tile([128, 512], bf16, tag="v2",
                                       name=f"v2{hp}{side}{jc}")
                        nc.vector.tensor_mul(v2[:], shf[:, sl], isg[:])
                        nc.vector.tensor_add(hat[:, sl], u[:], v2[:])
                    return go

                for jc in range(4):
                    items.append(mk_rope('q', jc))
                for jc in range(8):
                    items.append(mk_rope('k', jc))

                def mk_dup():
                    # duplicated per-head layouts for nh-packed scores
                    qh, kh = [], []
                    for h in range(2):
                        qt = pairq.tile([128, LQ], bf16, tag=f"qh{h}",
                                        name=f"qh{h}_{hp}")
                        kt = pairk.tile([128, L], bf16, tag=f"kh{h}",
                                        name=f"kh{h}_{hp}")
                        hsl = slice(h * 64, (h + 1) * 64)
                        nc.gpsimd.dma_start(qt[0:64, :], st['hatq'][hsl, :])
                        nc.gpsimd.dma_start(qt[64:128, :], st['hatq'][hsl, :])
                        nc.gpsimd.dma_start(kt[0:64, :], st['hatk'][hsl, :])
                        nc.gpsimd.dma_start(kt[64:128, :], st['hatk'][hsl, :])
                        qh.append(qt)
                        kh.append(kt)
                    st['qh'], st['kh'] = qh, kh
                items.append(mk_dup)

                return items, st

            # ------------------- out-projection work items ------------------
            def outproj_items(ip, tail=False):
                items = []
                ops = {}
                # at the pipeline tail the attention banks are idle too
                op_pools = ([(att, "att"), (att, "att"), (ppp, "pp"),
                             (ps2, "pss"), (cxp, "cxp")] if tail else
                            [(ppp, "pp"), (ps2, "pss")])
                ctr = [0]

                def mk_cb(ib, cb):
                    def go():
                        if cb == 0:
                            pools = [op_pools[(ctr[0] + k) % len(op_pools)]
                                     for k in range(2)]
                            ctr[0] += 2
                            ops[ib] = [
                                pools[0][0].tile([128, 512], f32,
                                                 tag=pools[0][1],
                                                 name=f"op0_{ib}"),
                                pools[1][0].tile([128, 512], f32,
                                                 tag=pools[1][1],
                                                 name=f"op1_{ib}"),
                            ]
                        ct = outp.tile([128, 128], bf16, tag="ct",
                                       name=f"ct{ib}_{cb}")
                        nc.sync.dma_start(
                            ct[:], ctx_sc[cb, :, ib * 128:(ib + 1) * 128])
                        for nh in range(2):
                            nc.tensor.matmul(
                                ops[ib][nh][:], ct[:],
                                wp_sb[:, cb, nh * 512:(nh + 1) * 512],
                                start=(cb == 0), stop=(cb == 7))
                    return go

                def mk_store(ib):
                    def go():
                        ot = otp.tile([128, C], f32, tag="ot", name=f"ot{ib}")
                        for nh in range(2):
                            nc.vector.tensor_add(
                                ot[:, nh * 512:(nh + 1) * 512],
                                ops[ib][nh][:],
                                bp_b[:, nh * 512:(nh + 1) * 512])
                        nc.sync.dma_start(out_ap[ib * 128:(ib + 1) * 128, :],
                                          ot[:])
                    return go

                for qb in range(8):
                    ib = ip * 8 + qb
                    items.extend(mk_cb(ib, cb) for cb in range(8))
                    items.append(mk_store(ib))
                return items

            # ---------------------- fused main pipeline ---------------------
            proj_queues = []  # list of pending work-item lists

            def drain_items(n):
                for _ in range(n):
                    for q in proj_queues:
                        if q:
                            q.pop(0)()
                            break
                    else:
                        return

            items0, st0 = proj_items(0, head=True)
            for it in items0:
                it()  # head: first projection runs dense
            cur = st0

            for hp in range(NPAIR):
                # enqueue next head-pair's projection (or out-proj for last)
                if hp + 1 < NPAIR:
                    nxt_items, nxt_st = proj_items(hp + 1)
                    proj_queues.append(nxt_items)
                else:
                    nxt_st = None

                total_slots = 128  # 2 ip * 2 h * 32 j
                for ip in range(2):
                    if hp == NPAIR - 1 and ip == 1:
                        proj_queues.append(outproj_items(0))
                    i0 = ip * 1024
                    for h in range(2):
                        ctxp = cxp.tile([65, 1024], f32, tag="cxp",
                                        name=f"cxp{hp}_{ip}_{h}")

                        def emit_av(jm, e):
                            for nh in range(2):
                                nc.tensor.matmul(
                                    ctxp[:, nh * 512:(nh + 1) * 512],
                                    cur['vsb'][:, jm, h, :],
                                    e[:, nh * 512:(nh + 1) * 512],
                                    start=(jm == 0), stop=(jm == 31))

                        prev_e = None
                        for j in range(32):
                            sc = att.tile([128, 1024], f32, tag="att",
                                          name=f"sc{hp}_{ip}_{h}_{j}")
                            for nh in range(2):
                                nc.tensor.matmul(
                                    sc[:, nh * 512:(nh + 1) * 512],
                                    cur['kh'][h][nh * 64:(nh + 1) * 64,
                                                 j * 128:(j + 1) * 128],
                                    cur['qh'][h][nh * 64:(nh + 1) * 64,
                                                 i0 + nh * 512:
                                                 i0 + (nh + 1) * 512],
                                    start=True, stop=True,
                                    tile_position=(nh * 64, 0))
                            e = exps.tile([128, 1024], bf16, tag="exps",
                                          name=f"e{hp}_{ip}_{h}_{j}")
                            nc.scalar.activation(e[:], sc[:], AF.Exp,
                                                 scale=0.125)
                            # AV lags one slot so PE never waits on ACT
                            if prev_e is not None:
                                emit_av(j - 1, prev_e)
                            prev_e = e
                            # drain interleaved projection work; front-load
                            # so nothing bunches at the hp boundary
                            rem = sum(len(q) for q in proj_queues)
                            slots_left = (total_slots
                                          - (ip * 64 + h * 32 + j))
                            eff = max(1, (slots_left * 6) // 10)
                            drain_items(-(-rem // eff))
                        emit_av(31, prev_e)

                        # ---- softmax normalize + store ctx ----
                        ctf = work1.tile([65, 1024], bf16, tag="ctf",
                                         name=f"ctf{hp}_{ip}_{h}")
                        nc.vector.tensor_copy(ctf[:], ctxp[:])
                        rcs = work1.tile([1, 1024], f32, tag="rcs",
                                         name=f"rcs{hp}_{ip}_{h}")
                        nc.vector.tensor_copy(rcs[:], ctf[64:65, :])
                        rcp = work1.tile([1, 1024], f32, tag="rcp",
                                         name=f"rcp{hp}_{ip}_{h}")
                        nc.vector.reciprocal_approx_fast(out=rcp[:],
                                                         in_=rcs[:])
                        rcpb = work1.tile([1, 1024], bf16, tag="rcpb",
                                          name=f"rcpb{hp}_{ip}_{h}")
                        nc.vector.tensor_copy(rcpb[:], rcp[:])
                        nc.sync.dma_start(rcp_sc[hp, ip, h:h + 1, :],
                                          rcpb[0:1, :])
                        rb = work1.tile([64, 1024], bf16, tag="rb",
                                        name=f"rb{hp}_{ip}_{h}")
                        nc.sync.dma_start(
                            rb[:],
                            rcp_sc[hp, ip, h:h + 1, :].partition_broadcast(64))
                        ctn = work1.tile([64, 1024], bf16, tag="ctn",
                                         name=f"ctn{hp}_{ip}_{h}")
                        nc.vector.tensor_mul(ctn[:], ctf[0:64, :], rb[:])
                        nc.sync.dma_start(
                            ctx_sc[hp, h * 64:(h + 1) * 64,
                                   i0:i0 + 1024], ctn[:])

                if nxt_st is not None:
                    # everything still pending must finish before next
                    # attention round uses it
                    drain_items(10 ** 9)
                    cur = nxt_st

            # tail: out-projection for ip=1 (+ anything left)
            drain_items(10 ** 9)
            for f in outproj_items(1, tail=True):
                f()

    nc.compile()
    return nc


def _host_prep(x, W_qkv, q_scale, k_scale, W_proj, b_proj, cos, sin):
    nbf = ml_dtypes.bfloat16
    cosn = np.asarray(cos, np.float32)
    sinn = np.asarray(sin, np.float32)
    qs = np.asarray(q_scale, np.float32)
    ks = np.asarray(k_scale, np.float32)

    def tables(g):
        sign = np.concatenate([-np.ones(D // 2), np.ones(D // 2)]).astype(np.float32)
        gpart = np.concatenate([g[D // 2:], g[:D // 2]])
        cg = cosn * g[None, :]
        sg = sinn * (sign * gpart)[None, :]
        return cg.T.copy(), sg.T.copy()

    cgq_f, sgq_f = tables(qs)
    cgk_f, sgk_f = tables(ks)

    wT = np.asarray(W_qkv, np.float32).T.astype(nbf)
    wpT = np.asarray(W_proj, np.float32).T.astype(nbf)
    bp = np.asarray(b_proj, np.float32).reshape(1, C)
    onesAB = np.zeros((128, 2), nbf)
    onesAB[0:64, 0] = 1.0
    onesAB[64:128, 1] = 1.0

    xn = np.asarray(x, np.float32)
    in_maps = []
    for core in range(NCORES):
        b, qh = core // 2, core % 2
        own = slice(qh * LQ, (qh + 1) * LQ)
        perm = np.r_[np.arange(qh * LQ, (qh + 1) * LQ),
                     np.arange((1 - qh) * LQ, (2 - qh) * LQ)]
        xTc = xn[b].T[:, perm].astype(nbf)
        in_maps.append({
            "xT": np.ascontiguousarray(xTc),
            "wT": wT, "wpT": wpT,
            "cgq": np.ascontiguousarray(cgq_f[:, own]).astype(nbf),
            "sgq": np.ascontiguousarray(sgq_f[:, own]).astype(nbf),
            "cgk": np.ascontiguousarray(cgk_f[:, perm]).astype(nbf),
            "sgk": np.ascontiguousarray(sgk_f[:, perm]).astype(nbf),
            "onesAB": onesAB, "bproj": bp,
        })
    return in_maps


def kernel(x, W_qkv, q_scale, k_scale, W_proj, b_proj, cos, sin, _trace=False):
    global _compiled
    from concourse.bass_utils import run_bass_kernel_spmd
    if _compiled is None:
        _compiled = _build()
    in_maps = _host_prep(x, W_qkv, q_scale, k_scale, W_proj, b_proj, cos, sin)
    res = run_bass_kernel_spmd(_compiled, in_maps, core_ids=list(range(NCORES)),
                               trace=_trace)
    out = np.empty((B, L, C), np.float32)
    for core in range(NCORES):
        b, qh = core // 2, core % 2
        out[b, qh * LQ:(qh + 1) * LQ, :] = res.results[core]["out"]
    kernel._last = res
    return out
